# revision 18
# baseline (speedup 1.0000x reference)
"""LocalVarianceMap Trainium2 kernel.

reference:
  lum  = mean over channel of x            (B,1,H,W)
  mean = 7x7 'same' box mean of lum ; sqm = same of lum^2
  out  = sqm - mean^2

Full input x: (16, 3, 1024, 1024) fp32. Data-parallel over batch:
8 NeuronCores x 2 images each.

Per-core pipeline per 128-row tile (partition=h, free=w), software-pipelined
with explicit emission skew so every engine queue head has >=1 tile of slack
(the engines dispatch in-order; mixing early/late stages of the same tile in
one queue serializes the whole pipeline):

  stage 0 (iter i):   SP: load x0 -> lum tile, load x2 -> X2 tile
  stage 1 (i-1):      GPSIMD: SWDGE accumulate-DMA x1 += into lum
  stage 2 (i-2):      GPSIMD: lum += X2 (tensor add); ACT: sq = lum^2
  stage 3 (i-3):      DVE: h1/h2 sliding 7-sum scans;
                      PE: S1 = band^T @ h1 (float32r), ACT: m2 = Square(S1/147),
                      PE: S2 = (-441 I) @ m2 (f32r) then += band^T @ h2 (fp32)
  stage 4 (i-4):      ACT: V = Copy(S2 / 441)
  stage 5 (i-5):      GPSIMD: SWDGE out-DMA (spreads across 16 SDMA engines)

Vertical tiling overlaps input tiles by 6 rows so each output tile's halo
lives inside one SBUF tile (single K<=128 banded matmul).
"""

import sys

if "/opt/trn_rl_repo" not in sys.path:
    sys.path.insert(0, "/opt/trn_rl_repo")

import numpy as np
from contextlib import ExitStack

import concourse.bass as bass
import concourse.bacc as bacc
import concourse.tile as tile
from concourse import mybir

H = 1024
W = 1024
C = 3
PER_CORE_B = 2
N_CORES = 8
K7 = 7
PADL, PADR = 7, 3
LW = PADL + W + PADR      # padded lum/sq width (1034)
SCAN_N = W + 3            # h[:, j+3] = centered 7-sum at col j
NRING = 6                 # lum ring depth (covers write i..i-2, read i-2..i-3)


def _tiles():
    specs = []
    for b in range(PER_CORE_B):
        specs.append(dict(b=b, r0=0, nr=128, K=128, M=125, out_r0=0, w=0))
        for t in range(1, 8):
            specs.append(
                dict(b=b, r0=122 * t, nr=128, K=128, M=122, out_r0=122 * t + 3, w=1)
            )
        specs.append(dict(b=b, r0=976, nr=48, K=48, M=45, out_r0=979, w=2))
    return specs


def band_weights() -> np.ndarray:
    """Four [128,128] blocks: W0 | Wmid | Wlast | -441*I."""
    wb = np.zeros((128, 4 * 128), np.float32)
    for m in range(125):
        for k in range(max(m - 3, 0), m + 4):
            wb[k, m] = 1.0
    for m in range(122):
        for k in range(m, m + 7):
            wb[k, 128 + m] = 1.0
    for m in range(45):
        for k in range(m, min(m + 7, 48)):
            wb[k, 256 + m] = 1.0
    for m in range(128):
        wb[m, 384 + m] = -441.0
    return wb


def build_nc(finalize: bool = True) -> bass.Bass:
    nc = bacc.Bacc("TRN2", target_bir_lowering=False)
    f32 = mybir.dt.float32
    f32r = mybir.dt.float32r

    x = nc.dram_tensor("x", [PER_CORE_B, C, H, W], f32, kind="ExternalInput")
    wbt = nc.dram_tensor("wb", [128, 4 * 128], f32r, kind="ExternalInput")
    y = nc.dram_tensor("y", [PER_CORE_B, 1, H, W], f32, kind="ExternalOutput")

    inv147 = float(np.float32(1.0) / np.float32(147.0))
    inv441 = float(np.float32(1.0) / np.float32(441.0))

    specs = _tiles()
    NT = len(specs)

    with tile.TileContext(nc) as tc, ExitStack() as ctx:
        cpool = ctx.enter_context(tc.tile_pool(name="const", bufs=1))
        xpool = ctx.enter_context(tc.tile_pool(name="x2", bufs=4))
        hpool = ctx.enter_context(tc.tile_pool(name="hsum", bufs=4))
        mpool = ctx.enter_context(tc.tile_pool(name="m2", bufs=3))
        vpool = ctx.enter_context(tc.tile_pool(name="vout", bufs=3))
        p1pool = ctx.enter_context(tc.tile_pool(name="ps1", bufs=2, space="PSUM"))
        p2pool = ctx.enter_context(tc.tile_pool(name="ps2", bufs=2, space="PSUM"))

        WB = cpool.tile([128, 4 * 128], f32r)
        nc.sync.dma_start(out=WB[:], in_=wbt[:, :])
        WBf = WB[:].bitcast(f32)

        # Persistent lum/sq rings; zero pads memset once.
        lum_ring = [
            cpool.tile([128, LW], f32, tag=f"lumr{i}", name=f"lumr{i}")
            for i in range(NRING)
        ]
        sq_ring = [
            cpool.tile([128, LW], f32, tag=f"sqr{i}", name=f"sqr{i}")
            for i in range(4)
        ]
        for t_ in lum_ring + sq_ring:
            nc.gpsimd.memset(t_[:, 0:PADL], 0.0)
            nc.gpsimd.memset(t_[:, PADL + W : LW], 0.0)

        X1s, X2s, H1s, H2s, M2s, S1s, S2s, Vs = {}, {}, {}, {}, {}, {}, {}, {}

        def st0_load(t):
            sp = specs[t]
            lum = lum_ring[t % NRING]
            nc.sync.dma_start(
                out=lum[0 : sp["nr"], PADL : PADL + W],
                in_=x[sp["b"], 0, sp["r0"] : sp["r0"] + sp["nr"], :],
            )
            X2 = xpool.tile([128, W], f32, tag="X2", name=f"X2_{t}")
            nc.sync.dma_start(
                out=X2[0 : sp["nr"], :],
                in_=x[sp["b"], 2, sp["r0"] : sp["r0"] + sp["nr"], :],
            )
            X2s[t] = X2
            if t % 2 == 1:
                X1 = xpool.tile([128, W], f32, tag="X1", name=f"X1_{t}")
                nc.sync.dma_start(
                    out=X1[0 : sp["nr"], :],
                    in_=x[sp["b"], 1, sp["r0"] : sp["r0"] + sp["nr"], :],
                )
                X1s[t] = X1

        def st1_acc(t):
            if t % 2 == 1:
                return
            sp = specs[t]
            lum = lum_ring[t % NRING]
            nc.gpsimd.dma_start(
                out=lum[0 : sp["nr"], PADL : PADL + W],
                in_=x[sp["b"], 1, sp["r0"] : sp["r0"] + sp["nr"], :],
                accum_op=mybir.AluOpType.add,
            )

        def st2_lum_sq(t):
            sp = specs[t]
            nr = sp["nr"]
            lum = lum_ring[t % NRING]
            nc.gpsimd.tensor_add(
                lum[0:nr, PADL : PADL + W],
                lum[0:nr, PADL : PADL + W],
                X2s.pop(t)[0:nr, :],
            )
            if t % 2 == 1:
                nc.gpsimd.tensor_add(
                    lum[0:nr, PADL : PADL + W],
                    lum[0:nr, PADL : PADL + W],
                    X1s.pop(t)[0:nr, :],
                )
            sq = sq_ring[t % 4]
            nc.scalar.activation(
                sq[0:nr, PADL : PADL + W],
                lum[0:nr, PADL : PADL + W],
                mybir.ActivationFunctionType.Square,
            )

        def st3_scan_mm(t):
            sp = specs[t]
            nr, K, M, wsel = sp["nr"], sp["K"], sp["M"], sp["w"]
            lum = lum_ring[t % NRING]
            sq = sq_ring[t % 4]
            h1 = hpool.tile([128, SCAN_N], f32r, tag="h1", name=f"h1_{t}")
            h2 = hpool.tile([128, SCAN_N], f32, tag="h2", name=f"h2_{t}")
            nc.vector.tensor_tensor_scan(
                out=h1[0:nr, :],
                data0=lum[0:nr, PADL : PADL + SCAN_N],
                data1=lum[0:nr, 0:SCAN_N],
                initial=0.0,
                op0=mybir.AluOpType.add,
                op1=mybir.AluOpType.subtract,
            )
            nc.vector.tensor_tensor_scan(
                out=h2[0:nr, :],
                data0=sq[0:nr, PADL : PADL + SCAN_N],
                data1=sq[0:nr, 0:SCAN_N],
                initial=0.0,
                op0=mybir.AluOpType.add,
                op1=mybir.AluOpType.subtract,
            )
            S1 = p1pool.tile([128, W], f32, tag="S1", name=f"S1_{t}")
            S2 = p2pool.tile([128, W], f32, tag="S2", name=f"S2_{t}")
            for cnk in range(2):
                nc.tensor.matmul(
                    S1[0:M, 512 * cnk : 512 * (cnk + 1)],
                    WB[0:K, 128 * wsel : 128 * wsel + M],
                    h1[0:K, 3 + 512 * cnk : 3 + 512 * (cnk + 1)],
                    start=True,
                    stop=True,
                )
            m2 = mpool.tile([128, W], f32, tag="m2", name=f"m2_{t}")
            nc.scalar.activation(
                m2[0:M, :],
                S1[0:M, :],
                mybir.ActivationFunctionType.Square,
                scale=inv147,
            )
            for cnk in range(2):
                nc.tensor.matmul(
                    S2[0:M, 512 * cnk : 512 * (cnk + 1)],
                    WBf[0:K, 128 * wsel : 128 * wsel + M],
                    h2[0:K, 3 + 512 * cnk : 3 + 512 * (cnk + 1)],
                    start=True,
                    stop=True,
                )
            H1s[t], H2s[t], M2s[t], S2s[t] = h1, h2, m2, S2

        def st4_copy(t):
            sp = specs[t]
            M = sp["M"]
            V = vpool.tile([128, W], f32, tag="V", name=f"V_{t}")
            nc.vector.scalar_tensor_tensor(
                out=V[0:M, :],
                in0=S2s.pop(t)[0:M, :],
                scalar=inv441,
                in1=M2s.pop(t)[0:M, :],
                op0=mybir.AluOpType.mult,
                op1=mybir.AluOpType.subtract,
            )
            Vs[t] = V

        def st5_out(t):
            sp = specs[t]
            M = sp["M"]
            nc.gpsimd.dma_start(
                out=y[sp["b"], 0, sp["out_r0"] : sp["out_r0"] + M, :],
                in_=Vs.pop(t)[0:M, :],
            )

        for i in range(NT + 5):
            if i < NT:
                st0_load(i)
            if 1 <= i < NT + 1:
                st1_acc(i - 1)
            if 5 <= i < NT + 5:
                st5_out(i - 5)
            if 2 <= i < NT + 2:
                st2_lum_sq(i - 2)
            if 4 <= i < NT + 4:
                st4_copy(i - 4)
            if 3 <= i < NT + 3:
                st3_scan_mm(i - 3)

    if finalize:
        nc.finalize()
    return nc


def kernel(x, kernel_size):
    assert int(kernel_size) == K7
    x = np.ascontiguousarray(np.asarray(x, dtype=np.float32))
    B = x.shape[0]
    assert x.shape == (B, C, H, W) and B == PER_CORE_B * N_CORES

    from concourse.bass_utils import run_bass_kernel_spmd

    nc = build_nc()
    wb = band_weights()
    in_maps = [
        {"x": x[i * PER_CORE_B : (i + 1) * PER_CORE_B], "wb": wb}
        for i in range(N_CORES)
    ]
    res = run_bass_kernel_spmd(nc, in_maps, list(range(N_CORES)))
    y = np.concatenate([res.results[i]["y"] for i in range(N_CORES)], axis=0)
    return y
